# revision 19
# baseline (speedup 1.0000x reference)
"""Trainium2 Bass kernel for nn_ChannelSELayerOwn (topk channel masking).

Reference computation (per batch sample b of 8, data-parallel across 8 cores):
  y   = mean(x[b], axis=(D,H,W))                       # (64,)
  h   = leaky_relu(w1 @ y + b1, 0.01)                  # (64,)
  z   = w2 @ h + b2                                    # (64,) pre-sigmoid logits
  idx = top_8 indices of sigmoid(z) == top_8 of z      # sigmoid is monotonic
  out[b] = x[b, idx]                                   # (8, D, H, W), bit-exact copy

Device kernel per core (one sample):
  phase A: stream x (viewed as 128 x 55296) through SBUF on the sync-engine
           HWDGE ring in program order -- a single ring saturates the
           ~440 GB/s aggregate HBM read cap (measured: a second ring or the
           SWDGE queue only steals from the same cap); each tile's
           column-sum is split between the DVE (reduce_sum) and the Act
           engine (activation Copy with accum_out) sized to their clock
           rates so each runs at ~45% duty and never lags the stream; FC1
           accumulates in PSUM as each unit's partials land (per-unit
           partial tiles -- a shared tile would create false WAR hazards in
           the tile-granular dependency tracker and serialize the stream);
           the last tile is split into 6 small sub-tiles so the final
           reduce adds <1us after the last load lands
  phase B: FC1 bias rides the PSUM chain as a K=1 matmul against a const
           one; leaky on DVE straight off PSUM; FC2 with b2 folded in as a
           65th weight row against an h vector extended by a const one;
           top-8 via DVE max8/max-index directly on the PSUM logits
           (sigmoid is monotonic); all matmuls fp32 (top-8/9 gaps ~1e-4)
  phase C: the 8 winning channels are copied HBM->HBM with plain DMAs
           using runtime (register) source offsets, spread over the sync
           and scalar HWDGE rings and the gpsimd SWDGE queue so the
           dispatch window stays short -- no SBUF staging
"""

import os
import time

import numpy as np

import concourse.bacc as bacc
import concourse.bass as bass
import concourse.mybir as mybir
from concourse import tile
from concourse.bass_utils import run_bass_kernel_spmd

F32 = mybir.dt.float32
U32 = mybir.dt.uint32

B, C, D, H, W = 8, 64, 48, 48, 48
M = D * H * W              # 110592 elements per channel
R_TOP = 8                  # channels kept
NEG_SLOPE = 0.01
N_CORES = 8

TF = 6912                  # streaming tile free-dim (55296 = 8 * 6912)
NT = (M * C // 128) // TF  # 8 full-size streaming tiles
TAIL_SPLIT = 6             # the last streaming tile is split this many ways
TFS = TF // TAIL_SPLIT     # 1152-wide sub-tiles for a short reduce tail

# DVE runs at 0.96 GHz, Act at 1.2 GHz (both 1 elem/cycle/partition):
# split each tile's columns so both finish together (Act pays a larger
# fixed SBUF-access cost, so small tiles tilt further toward DVE)
DVE_TF = 3200              # DVE columns per big tile
DVE_TFS = 640              # DVE columns per tail sub-tile

# results of the most recent run_bass_kernel_spmd call (for test harness use)
LAST_RESULTS = None
_NC_CACHE = None


def build_nc():
    nc = bacc.Bacc("TRN2", target_bir_lowering=False)

    x_d = nc.dram_tensor("x", [C, M], F32, kind="ExternalInput")
    w1r_d = nc.dram_tensor("w1r", [128, C], F32, kind="ExternalInput")
    b1r_d = nc.dram_tensor("b1r", [1, C], F32, kind="ExternalInput")
    w2te_d = nc.dram_tensor("w2te", [C + 1, C], F32, kind="ExternalInput")
    one1_d = nc.dram_tensor("one1", [1, 1], F32, kind="ExternalInput")
    out_d = nc.dram_tensor("out", [R_TOP, M], F32, kind="ExternalOutput")

    # x as 128 partitions x 55296: partition 2c+t holds half t of channel c
    x_stream = x_d[:].rearrange("c (t m) -> (c t) m", t=2)

    NUNIT = (NT - 1) + TAIL_SPLIT   # 13 streamed units

    with tile.TileContext(nc) as tc:
        with (
            tc.tile_pool(name="consts", bufs=1) as cpool,
            tc.tile_pool(name="stream", bufs=5) as spool,
            tc.tile_pool(name="small", bufs=1) as mpool,
            tc.tile_pool(name="psum", bufs=1, space="PSUM") as ppool,
        ):
            w1r = cpool.tile([128, C], F32)
            nc.scalar.dma_start(w1r[:], w1r_d[:])
            b1r = cpool.tile([1, C], F32)
            nc.scalar.dma_start(b1r[:], b1r_d[:])
            w2te = cpool.tile([C + 1, C], F32)
            nc.scalar.dma_start(w2te[:], w2te_d[:])
            one1 = cpool.tile([1, 1], F32)
            nc.scalar.dma_start(one1[:], one1_d[:])
            # h vector extended by a constant 1 so FC2's 65th weight row
            # (b2) adds the bias inside the matmul
            h_ext = mpool.tile([C + 1, 1], F32)
            nc.scalar.dma_start(h_ext[C : C + 1, :], one1_d[:])

            # ---- phase A: streaming channel sums ----
            ctxA = nc.named_scope("phaseA"); ctxA.__enter__()
            # one tiny tile PER UNIT per engine: the FC1 matmul reads a
            # unit's partial while the next unit's reduce writes its own --
            # separate tiles keep the tile-granular dependency tracker from
            # inventing WAR hazards that would serialize the stream
            partials_v = [
                mpool.tile([128, 1], F32, name=f"pv{u}") for u in range(NUNIT)
            ]
            partials_a = [
                mpool.tile([128, 1], F32, name=f"pa{u}") for u in range(NUNIT)
            ]
            # Act's activation needs a full-size main output; it is garbage
            # and reused every iteration (serializes Act with itself only)
            adump = mpool.tile([128, TF - DVE_TF], F32)

            # FC1 accumulates in PSUM as each unit's reduces land (PE is
            # idle during phase A anyway); after the last reduce only two
            # tiny matmuls remain on the critical path.  The b1 bias rides
            # the chain as a K=1 matmul against the const one.
            h_ps = ppool.tile([C, 1], F32)

            def unit_reduce(xt, cols, dcols, c):
                nc.vector.reduce_sum(
                    partials_v[c][:], xt[:, :dcols],
                    axis=mybir.AxisListType.X,
                )
                nc.scalar.activation(
                    adump[:, : cols - dcols], xt[:, dcols:cols],
                    mybir.ActivationFunctionType.Copy,
                    accum_out=partials_a[c][:],
                )
                nc.tensor.matmul(
                    h_ps[:], lhsT=w1r[:], rhs=partials_v[c][:],
                    start=(c == 0), stop=False,
                )
                if c == 0:
                    nc.tensor.matmul(
                        h_ps[:], lhsT=b1r[:], rhs=one1[:],
                        start=False, stop=False,
                    )
                nc.tensor.matmul(
                    h_ps[:], lhsT=w1r[:], rhs=partials_a[c][:],
                    start=False, stop=(c == NUNIT - 1),
                )

            col = 0
            for j in range(NT - 1):
                xt = spool.tile([128, TF], F32, tag="xt")
                nc.sync.dma_start(xt[:], x_stream[:, j * TF : (j + 1) * TF])
                unit_reduce(xt, TF, DVE_TF, col)
                col += 1
            base = (NT - 1) * TF
            for j in range(TAIL_SPLIT):
                xts = spool.tile([128, TFS], F32, tag="xts")
                nc.sync.dma_start(
                    xts[:], x_stream[:, base + j * TFS : base + (j + 1) * TFS]
                )
                unit_reduce(xts, TFS, DVE_TFS, col)
                col += 1

            ctxA.__exit__(None, None, None)
            # ---- phase B: leaky (PSUM already has h_pre + b1) -> FC2 -> top-8 ----
            ctxB = nc.named_scope("phaseB"); ctxB.__enter__()
            h_scaled = mpool.tile([C, 1], F32)
            nc.vector.tensor_scalar_mul(h_scaled[:], h_ps[:], NEG_SLOPE)
            nc.vector.tensor_tensor(
                h_ext[:C, :], h_ps[:], h_scaled[:], op=mybir.AluOpType.max
            )

            # z row with b2 folded in: [1,C] = h_ext[65,1].T @ w2te[65,C]
            zrow_ps = ppool.tile([1, C], F32)
            nc.tensor.matmul(zrow_ps[:], lhsT=h_ext[:], rhs=w2te[:], start=True, stop=True)

            m8 = mpool.tile([1, R_TOP], F32)
            nc.vector.max(m8[:], zrow_ps[:])
            idx8 = mpool.tile([1, R_TOP], U32)
            nc.vector.max_index(idx8[:], m8[:], zrow_ps[:])

            ctxB.__exit__(None, None, None)
            # ---- phase C: copy the selected channels HBM->HBM ----
            ctxC = nc.named_scope("phaseC"); ctxC.__enter__()
            _, idx_vals = nc.values_load_multi_w_load_instructions(
                idx8[:1, :],
                engines=[mybir.EngineType.SP, mybir.EngineType.Activation],
                min_val=0,
                max_val=C - 1,
                skip_runtime_bounds_check=True,
            )
            for r in range(R_TOP):
                eng = nc.sync if r % 2 == 0 else nc.scalar
                eng.dma_start(
                    out_d[r : r + 1, :], x_d[bass.ds(idx_vals[r], 1), :]
                )

            ctxC.__exit__(None, None, None)

    nc.compile()
    return nc


def _aux_inputs(w1, b1, w2, b2):
    # R[p, p//2] = 1/M so that R.T @ partition_sums = per-channel means
    rmat = np.zeros((128, C), dtype=np.float32)
    rmat[np.arange(128), np.arange(128) // 2] = np.float32(1.0 / M)
    return {
        "w1r": np.ascontiguousarray(rmat @ w1.T, dtype=np.float32),
        "b1r": np.ascontiguousarray(b1.reshape(1, C), dtype=np.float32),
        "w2te": np.ascontiguousarray(
            np.vstack([w2.T, b2.reshape(1, C)]), dtype=np.float32
        ),
        "one1": np.ones((1, 1), dtype=np.float32),
    }


def kernel(x, w1, b1, w2, b2):
    global LAST_RESULTS
    x = np.asarray(x, dtype=np.float32)
    aux = _aux_inputs(
        np.asarray(w1, np.float32), np.asarray(b1, np.float32),
        np.asarray(w2, np.float32), np.asarray(b2, np.float32),
    )
    global _NC_CACHE
    if _NC_CACHE is None:
        _NC_CACHE = build_nc()
    nc = _NC_CACHE
    in_maps = [
        {"x": np.ascontiguousarray(x[b].reshape(C, M)), **aux} for b in range(B)
    ]
    # the axon-tunneled device occasionally throws transient errors (e.g.
    # NRT_EXEC_UNIT_UNRECOVERABLE right after a fresh compile, or after an
    # earlier aborted run wedged it); pause briefly and retry
    res = None
    for attempt in range(4):
        try:
            res = run_bass_kernel_spmd(
                nc,
                in_maps,
                core_ids=list(range(N_CORES)),
                trace=bool(int(os.environ.get("BASS_PROFILE", "0"))),
            )
            break
        except Exception:
            if attempt == 3:
                raise
            time.sleep(10)
    LAST_RESULTS = res
    out = np.stack([res.results[b]["out"] for b in range(B)], axis=0)
    return out.reshape(B, R_TOP, D, H, W)


# revision 20
# speedup vs baseline: 1.0215x; 1.0215x over previous
"""Trainium2 Bass kernel for nn_ChannelSELayerOwn (topk channel masking).

Reference computation (per batch sample b of 8, data-parallel across 8 cores):
  y   = mean(x[b], axis=(D,H,W))                       # (64,)
  h   = leaky_relu(w1 @ y + b1, 0.01)                  # (64,)
  z   = w2 @ h + b2                                    # (64,) pre-sigmoid logits
  idx = top_8 indices of sigmoid(z) == top_8 of z      # sigmoid is monotonic
  out[b] = x[b, idx]                                   # (8, D, H, W), bit-exact copy

Device kernel per core (one sample):
  phase A: stream x (viewed as 128 x 55296) through SBUF on the sync-engine
           HWDGE ring in program order -- a single ring saturates the
           ~440 GB/s aggregate HBM read cap (measured: a second ring or the
           SWDGE queue only steals from the same cap); each tile's
           column-sum is split between the DVE (reduce_sum) and the Act
           engine (activation Copy with accum_out) sized to their clock
           rates so each runs at ~45% duty and never lags the stream; FC1
           accumulates in PSUM as each unit's partials land (per-unit
           partial tiles -- a shared tile would create false WAR hazards in
           the tile-granular dependency tracker and serialize the stream);
           the last tile is split into 6 small sub-tiles so the final
           reduce adds <1us after the last load lands
  phase B: FC1 bias rides the PSUM chain as a K=1 matmul against a const
           one; leaky on DVE straight off PSUM; FC2 with b2 folded in as a
           65th weight row against an h vector extended by a const one;
           top-8 via DVE max8/max-index directly on the PSUM logits
           (sigmoid is monotonic); all matmuls fp32 (top-8/9 gaps ~1e-4)
  phase C: the 8 winning channels are copied HBM->HBM with plain DMAs
           using runtime (register) source offsets, spread over the sync
           and scalar HWDGE rings and the gpsimd SWDGE queue so the
           dispatch window stays short -- no SBUF staging
"""

import os
import time

import numpy as np

import concourse.bacc as bacc
import concourse.bass as bass
import concourse.mybir as mybir
from concourse import tile
from concourse.bass_utils import run_bass_kernel_spmd

F32 = mybir.dt.float32
U32 = mybir.dt.uint32

B, C, D, H, W = 8, 64, 48, 48, 48
M = D * H * W              # 110592 elements per channel
R_TOP = 8                  # channels kept
NEG_SLOPE = 0.01
N_CORES = 8

TF = 6912                  # streaming tile free-dim (55296 = 8 * 6912)
NT = (M * C // 128) // TF  # 8 full-size streaming tiles
TAIL_SPLIT = 4             # the last streaming tile is split this many ways
TFS = TF // TAIL_SPLIT     # 1728-wide sub-tiles for a short reduce tail

# DVE runs at 0.96 GHz, Act at 1.2 GHz (both 1 elem/cycle/partition):
# split each tile's columns so both finish together (Act pays a larger
# fixed SBUF-access cost, so small tiles tilt further toward DVE)
DVE_TF = 3200              # DVE columns per big tile
DVE_TFS = 960              # DVE columns per tail sub-tile

# results of the most recent run_bass_kernel_spmd call (for test harness use)
LAST_RESULTS = None
_NC_CACHE = None


def build_nc():
    nc = bacc.Bacc("TRN2", target_bir_lowering=False)

    x_d = nc.dram_tensor("x", [C, M], F32, kind="ExternalInput")
    w1r_d = nc.dram_tensor("w1r", [128, C], F32, kind="ExternalInput")
    b1r_d = nc.dram_tensor("b1r", [1, C], F32, kind="ExternalInput")
    w2te_d = nc.dram_tensor("w2te", [C + 1, C], F32, kind="ExternalInput")
    one1_d = nc.dram_tensor("one1", [1, 1], F32, kind="ExternalInput")
    out_d = nc.dram_tensor("out", [R_TOP, M], F32, kind="ExternalOutput")

    # x as 128 partitions x 55296: partition 2c+t holds half t of channel c
    x_stream = x_d[:].rearrange("c (t m) -> (c t) m", t=2)

    NUNIT = (NT - 1) + TAIL_SPLIT   # 13 streamed units

    with tile.TileContext(nc) as tc:
        with (
            tc.tile_pool(name="consts", bufs=1) as cpool,
            tc.tile_pool(name="stream", bufs=5) as spool,
            tc.tile_pool(name="small", bufs=1) as mpool,
            tc.tile_pool(name="psum", bufs=1, space="PSUM") as ppool,
        ):
            w1r = cpool.tile([128, C], F32)
            nc.scalar.dma_start(w1r[:], w1r_d[:])
            b1r = cpool.tile([1, C], F32)
            nc.scalar.dma_start(b1r[:], b1r_d[:])
            w2te = cpool.tile([C + 1, C], F32)
            nc.scalar.dma_start(w2te[:], w2te_d[:])
            one1 = cpool.tile([1, 1], F32)
            nc.scalar.dma_start(one1[:], one1_d[:])
            # h vector extended by a constant 1 so FC2's 65th weight row
            # (b2) adds the bias inside the matmul
            h_ext = mpool.tile([C + 1, 1], F32)
            nc.scalar.dma_start(h_ext[C : C + 1, :], one1_d[:])

            # ---- phase A: streaming channel sums ----
            ctxA = nc.named_scope("phaseA"); ctxA.__enter__()
            # one tiny tile PER UNIT per engine: the FC1 matmul reads a
            # unit's partial while the next unit's reduce writes its own --
            # separate tiles keep the tile-granular dependency tracker from
            # inventing WAR hazards that would serialize the stream
            partials_v = [
                mpool.tile([128, 1], F32, name=f"pv{u}") for u in range(NUNIT)
            ]
            partials_a = [
                mpool.tile([128, 1], F32, name=f"pa{u}") for u in range(NUNIT)
            ]
            # Act's activation needs a full-size main output; it is garbage
            # and reused every iteration (serializes Act with itself only)
            adump = mpool.tile([128, TF - DVE_TF], F32)

            # FC1 accumulates in PSUM as each unit's reduces land (PE is
            # idle during phase A anyway); after the last reduce only two
            # tiny matmuls remain on the critical path.  The b1 bias rides
            # the chain as a K=1 matmul against the const one.
            h_ps = ppool.tile([C, 1], F32)

            def unit_reduce(xt, cols, dcols, c):
                nc.vector.reduce_sum(
                    partials_v[c][:], xt[:, :dcols],
                    axis=mybir.AxisListType.X,
                )
                nc.scalar.activation(
                    adump[:, : cols - dcols], xt[:, dcols:cols],
                    mybir.ActivationFunctionType.Copy,
                    accum_out=partials_a[c][:],
                )
                nc.tensor.matmul(
                    h_ps[:], lhsT=w1r[:], rhs=partials_v[c][:],
                    start=(c == 0), stop=False,
                )
                if c == 0:
                    nc.tensor.matmul(
                        h_ps[:], lhsT=b1r[:], rhs=one1[:],
                        start=False, stop=False,
                    )
                nc.tensor.matmul(
                    h_ps[:], lhsT=w1r[:], rhs=partials_a[c][:],
                    start=False, stop=(c == NUNIT - 1),
                )

            col = 0
            for j in range(NT - 1):
                xt = spool.tile([128, TF], F32, tag="xt")
                nc.sync.dma_start(xt[:], x_stream[:, j * TF : (j + 1) * TF])
                unit_reduce(xt, TF, DVE_TF, col)
                col += 1
            base = (NT - 1) * TF
            for j in range(TAIL_SPLIT):
                xts = spool.tile([128, TFS], F32, tag="xts")
                nc.sync.dma_start(
                    xts[:], x_stream[:, base + j * TFS : base + (j + 1) * TFS]
                )
                unit_reduce(xts, TFS, DVE_TFS, col)
                col += 1

            ctxA.__exit__(None, None, None)
            # ---- phase B: leaky (PSUM already has h_pre + b1) -> FC2 -> top-8 ----
            ctxB = nc.named_scope("phaseB"); ctxB.__enter__()
            h_scaled = mpool.tile([C, 1], F32)
            nc.vector.tensor_scalar_mul(h_scaled[:], h_ps[:], NEG_SLOPE)
            nc.vector.tensor_tensor(
                h_ext[:C, :], h_ps[:], h_scaled[:], op=mybir.AluOpType.max
            )

            # z row with b2 folded in: [1,C] = h_ext[65,1].T @ w2te[65,C]
            zrow_ps = ppool.tile([1, C], F32)
            nc.tensor.matmul(zrow_ps[:], lhsT=h_ext[:], rhs=w2te[:], start=True, stop=True)

            m8 = mpool.tile([1, R_TOP], F32)
            nc.vector.max(m8[:], zrow_ps[:])
            idx8 = mpool.tile([1, R_TOP], U32)
            nc.vector.max_index(idx8[:], m8[:], zrow_ps[:])

            ctxB.__exit__(None, None, None)
            # ---- phase C: copy the selected channels HBM->HBM ----
            ctxC = nc.named_scope("phaseC"); ctxC.__enter__()
            _, idx_vals = nc.values_load_multi_w_load_instructions(
                idx8[:1, :],
                engines=[mybir.EngineType.SP, mybir.EngineType.Activation],
                min_val=0,
                max_val=C - 1,
                skip_runtime_bounds_check=True,
            )
            for r in range(R_TOP):
                eng = nc.sync if r % 2 == 0 else nc.scalar
                eng.dma_start(
                    out_d[r : r + 1, :], x_d[bass.ds(idx_vals[r], 1), :]
                )

            ctxC.__exit__(None, None, None)

    nc.compile()
    return nc


def _aux_inputs(w1, b1, w2, b2):
    # R[p, p//2] = 1/M so that R.T @ partition_sums = per-channel means
    rmat = np.zeros((128, C), dtype=np.float32)
    rmat[np.arange(128), np.arange(128) // 2] = np.float32(1.0 / M)
    return {
        "w1r": np.ascontiguousarray(rmat @ w1.T, dtype=np.float32),
        "b1r": np.ascontiguousarray(b1.reshape(1, C), dtype=np.float32),
        "w2te": np.ascontiguousarray(
            np.vstack([w2.T, b2.reshape(1, C)]), dtype=np.float32
        ),
        "one1": np.ones((1, 1), dtype=np.float32),
    }


def kernel(x, w1, b1, w2, b2):
    global LAST_RESULTS
    x = np.asarray(x, dtype=np.float32)
    aux = _aux_inputs(
        np.asarray(w1, np.float32), np.asarray(b1, np.float32),
        np.asarray(w2, np.float32), np.asarray(b2, np.float32),
    )
    global _NC_CACHE
    if _NC_CACHE is None:
        _NC_CACHE = build_nc()
    nc = _NC_CACHE
    in_maps = [
        {"x": np.ascontiguousarray(x[b].reshape(C, M)), **aux} for b in range(B)
    ]
    # the axon-tunneled device occasionally throws transient errors (e.g.
    # NRT_EXEC_UNIT_UNRECOVERABLE right after a fresh compile, or after an
    # earlier aborted run wedged it); pause briefly and retry
    res = None
    for attempt in range(4):
        try:
            res = run_bass_kernel_spmd(
                nc,
                in_maps,
                core_ids=list(range(N_CORES)),
                trace=bool(int(os.environ.get("BASS_PROFILE", "0"))),
            )
            break
        except Exception:
            if attempt == 3:
                raise
            time.sleep(10)
    LAST_RESULTS = res
    out = np.stack([res.results[b]["out"] for b in range(B)], axis=0)
    return out.reshape(B, R_TOP, D, H, W)


# revision 21
# speedup vs baseline: 1.1426x; 1.1186x over previous
"""Trainium2 Bass kernel for nn_ChannelSELayerOwn (topk channel masking).

Reference computation (per batch sample b of 8, data-parallel across 8 cores):
  y   = mean(x[b], axis=(D,H,W))                       # (64,)
  h   = leaky_relu(w1 @ y + b1, 0.01)                  # (64,)
  z   = w2 @ h + b2                                    # (64,) pre-sigmoid logits
  idx = top_8 indices of sigmoid(z) == top_8 of z      # sigmoid is monotonic
  out[b] = x[b, idx]                                   # (8, D, H, W), bit-exact copy

Device kernel per core (one sample):
  phase A: stream x (viewed as 128 x 55296) through SBUF on the sync-engine
           HWDGE ring in program order -- a single ring saturates the
           ~440 GB/s aggregate HBM read cap (measured: a second ring or the
           SWDGE queue only steals from the same cap); each tile's
           column-sum is split between the DVE (reduce_sum) and the Act
           engine (activation Copy with accum_out) sized to their clock
           rates so each runs at ~45% duty and never lags the stream; FC1
           accumulates in PSUM as each unit's partials land (per-unit
           partial tiles -- a shared tile would create false WAR hazards in
           the tile-granular dependency tracker and serialize the stream);
           the last tile is split into 6 small sub-tiles so the final
           reduce adds <1us after the last load lands
  phase B: FC1 bias rides the PSUM chain as a K=1 matmul against a const
           one; leaky on DVE straight off PSUM; FC2 with b2 folded in as a
           65th weight row against an h vector extended by a const one;
           top-8 via DVE max8/max-index directly on the PSUM logits
           (sigmoid is monotonic); all matmuls fp32 (top-8/9 gaps ~1e-4)
  phase C: the 8 winning channels are copied HBM->HBM with plain DMAs
           using runtime (register) source offsets, spread over the sync
           and scalar HWDGE rings and the gpsimd SWDGE queue so the
           dispatch window stays short -- no SBUF staging
"""

import os
import time

import numpy as np

import concourse.bacc as bacc
import concourse.bass as bass
import concourse.mybir as mybir
from concourse import tile
from concourse.bass_utils import run_bass_kernel_spmd

F32 = mybir.dt.float32
U32 = mybir.dt.uint32

B, C, D, H, W = 8, 64, 48, 48, 48
M = D * H * W              # 110592 elements per channel
R_TOP = 8                  # channels kept
NEG_SLOPE = 0.01
N_CORES = 8

TF = 13824                 # streaming tile free-dim (55296 = 4 * 13824)
NT = (M * C // 128) // TF  # 4 full-size streaming tiles
TAIL_SPLIT = 4             # the last streaming tile is split this many ways
TFS = TF // TAIL_SPLIT     # 3456-wide sub-tiles for a short reduce tail

# DVE runs at 0.96 GHz, Act at 1.2 GHz (both 1 elem/cycle/partition):
# split each tile's columns so both finish together (Act pays a larger
# fixed SBUF-access cost, so small tiles tilt further toward DVE)
DVE_TF = 6486              # DVE columns per big tile
DVE_TFS = 1601             # DVE columns per tail sub-tile

# results of the most recent run_bass_kernel_spmd call (for test harness use)
LAST_RESULTS = None
_NC_CACHE = None


def build_nc():
    nc = bacc.Bacc("TRN2", target_bir_lowering=False)

    x_d = nc.dram_tensor("x", [C, M], F32, kind="ExternalInput")
    w1r_d = nc.dram_tensor("w1r", [128, C], F32, kind="ExternalInput")
    b1r_d = nc.dram_tensor("b1r", [1, C], F32, kind="ExternalInput")
    w2te_d = nc.dram_tensor("w2te", [C + 1, C], F32, kind="ExternalInput")
    one1_d = nc.dram_tensor("one1", [1, 1], F32, kind="ExternalInput")
    out_d = nc.dram_tensor("out", [R_TOP, M], F32, kind="ExternalOutput")

    # x as 128 partitions x 55296: partition 2c+t holds half t of channel c
    x_stream = x_d[:].rearrange("c (t m) -> (c t) m", t=2)

    NUNIT = (NT - 1) + TAIL_SPLIT   # 13 streamed units

    with tile.TileContext(nc) as tc:
        with (
            tc.tile_pool(name="consts", bufs=1) as cpool,
            tc.tile_pool(name="stream", bufs=2) as spool,
            tc.tile_pool(name="small", bufs=1) as mpool,
            tc.tile_pool(name="psum", bufs=1, space="PSUM") as ppool,
        ):
            w1r = cpool.tile([128, C], F32)
            nc.scalar.dma_start(w1r[:], w1r_d[:])
            b1r = cpool.tile([1, C], F32)
            nc.scalar.dma_start(b1r[:], b1r_d[:])
            w2te = cpool.tile([C + 1, C], F32)
            nc.scalar.dma_start(w2te[:], w2te_d[:])
            one1 = cpool.tile([1, 1], F32)
            nc.scalar.dma_start(one1[:], one1_d[:])
            # h vector extended by a constant 1 so FC2's 65th weight row
            # (b2) adds the bias inside the matmul
            h_ext = mpool.tile([C + 1, 1], F32)
            nc.scalar.dma_start(h_ext[C : C + 1, :], one1_d[:])

            # ---- phase A: streaming channel sums ----
            ctxA = nc.named_scope("phaseA"); ctxA.__enter__()
            # one tiny tile PER UNIT per engine: the FC1 matmul reads a
            # unit's partial while the next unit's reduce writes its own --
            # separate tiles keep the tile-granular dependency tracker from
            # inventing WAR hazards that would serialize the stream
            partials_v = [
                mpool.tile([128, 1], F32, name=f"pv{u}") for u in range(NUNIT)
            ]
            partials_a = [
                mpool.tile([128, 1], F32, name=f"pa{u}") for u in range(NUNIT)
            ]
            # Act's activation needs a full-size main output; it is garbage
            # and reused every iteration (serializes Act with itself only)
            adump = mpool.tile([128, TF - DVE_TF], F32)

            # FC1 accumulates in PSUM as each unit's reduces land (PE is
            # idle during phase A anyway); after the last reduce only two
            # tiny matmuls remain on the critical path.  The b1 bias rides
            # the chain as a K=1 matmul against the const one.
            h_ps = ppool.tile([C, 1], F32)

            def unit_reduce(xt, cols, dcols, c):
                nc.vector.reduce_sum(
                    partials_v[c][:], xt[:, :dcols],
                    axis=mybir.AxisListType.X,
                )
                nc.scalar.activation(
                    adump[:, : cols - dcols], xt[:, dcols:cols],
                    mybir.ActivationFunctionType.Copy,
                    accum_out=partials_a[c][:],
                )
                nc.tensor.matmul(
                    h_ps[:], lhsT=w1r[:], rhs=partials_v[c][:],
                    start=(c == 0), stop=False,
                )
                if c == 0:
                    nc.tensor.matmul(
                        h_ps[:], lhsT=b1r[:], rhs=one1[:],
                        start=False, stop=False,
                    )
                nc.tensor.matmul(
                    h_ps[:], lhsT=w1r[:], rhs=partials_a[c][:],
                    start=False, stop=(c == NUNIT - 1),
                )

            col = 0
            for j in range(NT - 1):
                xt = spool.tile([128, TF], F32, tag="xt")
                nc.sync.dma_start(xt[:], x_stream[:, j * TF : (j + 1) * TF])
                unit_reduce(xt, TF, DVE_TF, col)
                col += 1
            base = (NT - 1) * TF
            for j in range(TAIL_SPLIT):
                xts = spool.tile([128, TFS], F32, tag="xts")
                nc.sync.dma_start(
                    xts[:], x_stream[:, base + j * TFS : base + (j + 1) * TFS]
                )
                unit_reduce(xts, TFS, DVE_TFS, col)
                col += 1

            ctxA.__exit__(None, None, None)
            # ---- phase B: leaky (PSUM already has h_pre + b1) -> FC2 -> top-8 ----
            ctxB = nc.named_scope("phaseB"); ctxB.__enter__()
            h_scaled = mpool.tile([C, 1], F32)
            nc.vector.tensor_scalar_mul(h_scaled[:], h_ps[:], NEG_SLOPE)
            nc.vector.tensor_tensor(
                h_ext[:C, :], h_ps[:], h_scaled[:], op=mybir.AluOpType.max
            )

            # z row with b2 folded in: [1,C] = h_ext[65,1].T @ w2te[65,C]
            zrow_ps = ppool.tile([1, C], F32)
            nc.tensor.matmul(zrow_ps[:], lhsT=h_ext[:], rhs=w2te[:], start=True, stop=True)

            m8 = mpool.tile([1, R_TOP], F32)
            nc.vector.max(m8[:], zrow_ps[:])
            idx8 = mpool.tile([1, R_TOP], U32)
            nc.vector.max_index(idx8[:], m8[:], zrow_ps[:])

            ctxB.__exit__(None, None, None)
            # ---- phase C: copy the selected channels HBM->HBM ----
            ctxC = nc.named_scope("phaseC"); ctxC.__enter__()
            _, idx_vals = nc.values_load_multi_w_load_instructions(
                idx8[:1, :],
                engines=[mybir.EngineType.SP, mybir.EngineType.Activation],
                min_val=0,
                max_val=C - 1,
                skip_runtime_bounds_check=True,
            )
            for r in range(R_TOP):
                eng = nc.sync if r % 2 == 0 else nc.scalar
                eng.dma_start(
                    out_d[r : r + 1, :], x_d[bass.ds(idx_vals[r], 1), :]
                )

            ctxC.__exit__(None, None, None)

    nc.compile()
    return nc


def _aux_inputs(w1, b1, w2, b2):
    # R[p, p//2] = 1/M so that R.T @ partition_sums = per-channel means
    rmat = np.zeros((128, C), dtype=np.float32)
    rmat[np.arange(128), np.arange(128) // 2] = np.float32(1.0 / M)
    return {
        "w1r": np.ascontiguousarray(rmat @ w1.T, dtype=np.float32),
        "b1r": np.ascontiguousarray(b1.reshape(1, C), dtype=np.float32),
        "w2te": np.ascontiguousarray(
            np.vstack([w2.T, b2.reshape(1, C)]), dtype=np.float32
        ),
        "one1": np.ones((1, 1), dtype=np.float32),
    }


def kernel(x, w1, b1, w2, b2):
    global LAST_RESULTS
    x = np.asarray(x, dtype=np.float32)
    aux = _aux_inputs(
        np.asarray(w1, np.float32), np.asarray(b1, np.float32),
        np.asarray(w2, np.float32), np.asarray(b2, np.float32),
    )
    global _NC_CACHE
    if _NC_CACHE is None:
        _NC_CACHE = build_nc()
    nc = _NC_CACHE
    in_maps = [
        {"x": np.ascontiguousarray(x[b].reshape(C, M)), **aux} for b in range(B)
    ]
    # the axon-tunneled device occasionally throws transient errors (e.g.
    # NRT_EXEC_UNIT_UNRECOVERABLE right after a fresh compile, or after an
    # earlier aborted run wedged it); pause briefly and retry
    res = None
    for attempt in range(4):
        try:
            res = run_bass_kernel_spmd(
                nc,
                in_maps,
                core_ids=list(range(N_CORES)),
                trace=bool(int(os.environ.get("BASS_PROFILE", "0"))),
            )
            break
        except Exception:
            if attempt == 3:
                raise
            time.sleep(10)
    LAST_RESULTS = res
    out = np.stack([res.results[b]["out"] for b in range(B)], axis=0)
    return out.reshape(B, R_TOP, D, H, W)
